# revision 1
# baseline (speedup 1.0000x reference)
"""Trainium2 Bass kernel for a 4-layer IndRNN (B=32, T=2048, I=256, H=512).

Math: per layer, xp = x @ W.T + b, then the per-channel recurrence
    h_t = relu(xp_t + w * h_{t-1}),  w = whs[l] in [0, 1)

Since w >= 0, the nonlinear scan decomposes into two linear-style scans that
map 1:1 onto the DVE `tensor_tensor_scan` instruction:
    dloc_t = w * dloc_{t-1} + xp_t          (unclamped linear scan)
    q_t    = w * min(dloc_{t-1}, q_{t-1})   (min-scan; dloc_{-1} = q_{-1} = 0)
    h_t    = relu(dloc_t - q_t)
Proof sketch: with s_t = xp_t + w*relu(s_{t-1}) (so h_t = relu(s_t)),
s_t = max(xp_t, xp_t + w*s_{t-1}) for w >= 0; substituting s_t = dloc_t + r_t
gives r_t = max(-w*dloc_{t-1}, w*r_{t-1}), i.e. q_t = -r_t satisfies the
min-scan above. Verified exactly in fp64 against the sequential reference.

Sharding: data-parallel over batch, 4 batches per core, weights replicated.
Layout on device: [H(partitions), T(free)] per batch; the host pre-transposes
the layer-0 input to [I, T] and post-transposes the output from [H, T], so the
device never pays for transposes.
"""

import numpy as np
from contextlib import ExitStack

import concourse.bass as bass
import concourse.tile as tile
from concourse import mybir
from concourse.bass_utils import run_bass_kernel_spmd

dt = mybir.dt
Alu = mybir.AluOpType
Act = mybir.ActivationFunctionType

B, T, I, H, L = 32, 2048, 256, 512, 4
NCORES = 8
BLOC = B // NCORES
P = 128
TCH = 512  # time chunk = one PSUM bank of fp32


def build(bloc=BLOC, t=T, include_bias=False, trace_sim=False,
          whole_t_scan=True, bcast_w=True, gpsimd_stt=False, act_relu=True):
    """Build the per-core Bass program (SPMD; identical on all cores)."""
    assert t % TCH == 0
    nch = t // TCH
    ki, kh, m4 = I // P, H // P, H // P

    nc = bass.Bass("TRN2", target_bir_lowering=False, debug=False,
                   num_devices=NCORES)
    xT_d = nc.dram_tensor("xT", [bloc, I, t], dt.float16, kind="ExternalInput").ap()
    w0t_d = nc.dram_tensor("w0t", [I, H], dt.float16, kind="ExternalInput").ap()
    wst_d = nc.dram_tensor("wst", [L - 1, H, H], dt.float16, kind="ExternalInput").ap()
    bias_d = nc.dram_tensor("bias", [L, 1, H], dt.float16, kind="ExternalInput").ap()
    wbc_d = nc.dram_tensor("wbc", [L, H, TCH], dt.float32, kind="ExternalInput").ap()
    out_d = nc.dram_tensor("out", [bloc, H, t], dt.float16, kind="ExternalOutput").ap()

    with tile.TileContext(nc, trace_sim=trace_sim) as tc, ExitStack() as ctx:
        wpool = ctx.enter_context(tc.tile_pool(name="weights", bufs=1))
        xpool = ctx.enter_context(tc.tile_pool(name="xin", bufs=2 * BLOC))
        hpool = ctx.enter_context(tc.tile_pool(name="h", bufs=8))
        dpool = ctx.enter_context(tc.tile_pool(name="dloc", bufs=3))
        qpool = ctx.enter_context(tc.tile_pool(name="q", bufs=3))
        spool = ctx.enter_context(tc.tile_pool(name="s", bufs=3))
        opool = ctx.enter_context(tc.tile_pool(name="hout", bufs=BLOC * (H // P) // 2))
        psum = ctx.enter_context(tc.tile_pool(name="psum", bufs=2, space="PSUM"))

        # --- persistent weights ---
        # lhsT tiles [K=128, M<=512]; lhsT slice [:, m*128:(m+1)*128] per matmul
        wt = []  # wt[l][k] -> [128, H] fp16
        for l in range(L):
            kprev = ki if l == 0 else kh
            tiles = []
            for k in range(kprev):
                w = wpool.tile([P, H], dt.float16, tag=f"w{l}{k}")
                src = w0t_d[k * P:(k + 1) * P, :] if l == 0 else \
                    wst_d[l - 1, k * P:(k + 1) * P, :]
                nc.gpsimd.dma_start(out=w[:], in_=src)
                tiles.append(w)
            wt.append(tiles)
        wbc = []  # wbc[l][m] -> [128, TCH or 1] fp32 recurrent weight
        wb_width = 1 if bcast_w else TCH
        for l in range(L):
            tiles = []
            for m in range(m4):
                w = wpool.tile([P, wb_width], dt.float32, tag=f"wb{l}{m}")
                nc.gpsimd.dma_start(
                    out=w[:], in_=wbc_d[l, m * P:(m + 1) * P, 0:wb_width])
                tiles.append(w)
            wbc.append(tiles)
        if include_bias:
            bias = []
            for l in range(L):
                bt = wpool.tile([1, H], dt.float16, tag=f"b{l}")
                nc.gpsimd.dma_start(out=bt[:], in_=bias_d[l, :, :])
                bias.append(bt)
            ones = wpool.tile([1, TCH], dt.float16, tag="ones")
            nc.gpsimd.memset(ones[:], 1.0)
        # Non-PE instructions can carry only ONE sync-wait through walrus
        # codegen (probed: DVE scan/copy and ACT activation all fail with 2).
        # Same-engine waits merge into one semaphore, so the scheme is:
        # each engine touches every cross-engine dependency in a cheap
        # "absorber/claimer" op first, leaving the real op a single wait.
        # Preamble: DVE and ACT each touch every DMA-loaded scan operand so
        # later ops never need a DMA-queue wait.
        scratch = wpool.tile([P, L * m4], dt.float32, tag="scratch")
        scr_act = wpool.tile([P, L * m4], dt.float32, tag="scr_act")
        for l in range(L):
            for m in range(m4):
                col = slice(l * m4 + m, l * m4 + m + 1)
                nc.vector.tensor_copy(scratch[:, col], wbc[l][m][:, 0:1])
                nc.scalar.activation(scr_act[:, col], wbc[l][m][:, 0:1],
                                     Act.Relu)
        # rotating per-tile scratch columns for the ACT claimer chain (a
        # fixed column would WAW against itself and add an ACT-own wait on
        # top of the DVE data wait)
        scr_rot = wpool.tile([P, 2 * bloc * L * m4], dt.float32, tag="scr_rot")
        scr_gp = wpool.tile([P, bloc * m4], dt.float32, tag="scr_gp")
        # PE preamble: junk ldweights per weight tile (no PSUM write, so no
        # WAW) so later real matmuls never carry a weight-DMA wait (PE is
        # also a 1-sync-wait engine).
        for l in range(L):
            for k in range(len(wt[l])):
                nc.tensor.ldweights(weights=wt[l][k][:, 0:P])
        if include_bias:
            for l in range(L):
                nc.tensor.ldweights(weights=bias[l][:, 0:P])
            nc.tensor.ldweights(weights=ones[:, 0:P])

        # --- main loop ---
        houts = {}
        xp_count = 0
        xp_readers = {}  # psum slot -> last scan1 instruction that read it
        for b in range(bloc):
            xtiles = []
            for k in range(ki):
                xt = xpool.tile([P, t], dt.float16, tag="xin")
                nc.gpsimd.dma_start(out=xt[:], in_=xT_d[b, k * P:(k + 1) * P, :])
                xtiles.append(xt)
            prev = xtiles
            for l in range(L):
                htiles = []
                for m in range(m4):
                    xp = psum.tile([P, t], dt.float32, tag="xp")
                    kprev = len(prev)
                    # PE claimer ldweights (junk loads, no PSUM write): one
                    # absorbs the DVE scan tick guarding the recycled PSUM
                    # slot (forced dep), the m==0 extras absorb the rhs
                    # producer tick (input DMA for layer 0, ACT relu after).
                    old_rd = xp_readers.get(xp_count % 2)
                    xp_count += 1
                    claimers = []
                    if old_rd is not None:
                        ldw = nc.tensor.ldweights(weights=wt[l][0][:, 0:P])
                        bass._add_dep_helper(
                            ldw.ins, old_rd.ins, sync=True,
                            reason="PE DVE-clock claimer for PSUM slot WAR")
                        claimers.append(ldw)
                    if m == 0:
                        for kc in range(kprev if l == 0 else 1):
                            claimers.append(nc.tensor.ldweights(
                                weights=prev[kprev - 1 - kc][:, 0:P]))
                    last_mm = None
                    for n in range(nch):
                        ns = slice(n * TCH, (n + 1) * TCH)
                        for k in range(kprev):
                            last_mm = nc.tensor.matmul(
                                xp[:, ns], lhsT=wt[l][k][:, m * P:(m + 1) * P],
                                rhs=prev[k][:, ns],
                                start=(k == 0),
                                stop=(k == kprev - 1 and not include_bias))
                            for cl in claimers:  # pin claimers before 1st MM
                                bass._add_dep_helper(
                                    last_mm.ins, cl.ins, sync=False,
                                    reason="order claimer before real MMs")
                            claimers = []
                        if include_bias:
                            last_mm = nc.tensor.matmul(
                                xp[:, ns], lhsT=bias[l][:, m * P:(m + 1) * P],
                                rhs=ones[:, :], start=False, stop=True)
                    # dlocb[i+1] = dloc_i, dlocb[0] = dloc_{-1} = 0.
                    # The [128,1] DVE memsets below are slot "claimers": the
                    # first toucher of a recycled pool tile carries the
                    # cross-engine WAR wait, keeping the scan/stt instructions
                    # (whose ISA struct fits a single sync-wait) at <=1 wait.
                    dlocb = dpool.tile([P, t + 2], dt.float16, tag="dloc")
                    nc.vector.memset(dlocb[:, 0:2], 0.0)
                    q = qpool.tile([P, t], dt.float16, tag="q")
                    if bcast_w:
                        wb_full = wbc[l][m][:, 0:1].broadcast_to((P, t))
                    if whole_t_scan:
                        scan_chunks = [(0, t)]
                    else:
                        scan_chunks = [(c * TCH, (c + 1) * TCH) for c in range(nch)]
                    for cs, ce in scan_chunks:
                        wb = wb_full[:, 0:ce - cs] if bcast_w else wbc[l][m][:]
                        scan1 = nc.vector.tensor_tensor_scan(
                            out=dlocb[:, cs + 2:ce + 2],
                            data0=wb, data1=xp[:, cs:ce],
                            initial=(0.0 if cs == 0 else dlocb[:, cs + 1:cs + 2]),
                            op0=Alu.mult, op1=Alu.add)
                    xp_readers[(xp_count - 1) % 2] = scan1
                    for cs, ce in scan_chunks:
                        wb = wb_full[:, 0:ce - cs] if bcast_w else wbc[l][m][:]
                        nc.vector.tensor_tensor_scan(
                            out=q[:, cs:ce],
                            data0=dlocb[:, cs + 1:ce + 1], data1=wb,
                            initial=(0.0 if cs == 0 else q[:, cs - 1:cs]),
                            op0=Alu.min, op1=Alu.mult)
                    s = spool.tile([P, t], dt.float16, tag="s")
                    nc.vector.memset(s[:, 0:1], 0.0)
                    stt_eng = nc.gpsimd if gpsimd_stt else nc.vector
                    stt_eng.scalar_tensor_tensor(
                        out=s[:], in0=q[:], scalar=-1.0, in1=dlocb[:, 2:t + 2],
                        op0=Alu.mult, op1=Alu.add)
                    ti2 = 2 * ((b * L + l) * m4 + m)
                    if l < L - 1:
                        # ACT claimer chain: claimer0 absorbs the PE tick
                        # that guards the recycled h slot (forced via an
                        # explicit dep on this tile's last matmul), claimer1
                        # absorbs the DVE tick for s; the relu itself is then
                        # left with a single ACT-own ordering wait.
                        h = hpool.tile([P, t], dt.float16, tag="h")
                        c0 = nc.scalar.activation(scr_rot[:, ti2:ti2 + 1],
                                                  wbc[l][m][:, 0:1], Act.Relu)
                        bass._add_dep_helper(
                            c0.ins, last_mm.ins, sync=True,
                            reason="ACT PE-clock claimer for h slot WAR")
                        nc.scalar.activation(scr_rot[:, ti2 + 1:ti2 + 2],
                                             s[:, 0:1], Act.Relu)
                        nc.scalar.activation(h[:], s[:], Act.Relu)
                        htiles.append(h)
                    else:
                        # Final layer on DVE. Outputs for two consecutive
                        # batches share one [P, 2t] tile and go out in ONE
                        # DMA (8 stores total = one per HWDGE queue, so no
                        # same-queue flow-control wait, keeping each store
                        # at its single DVE data wait).
                        if b % 2 == 0:
                            h2 = opool.tile([P, 2 * t], dt.float16,
                                            tag="hout")
                            houts[m] = h2
                            nc.vector.memset(h2[:, 0:1], 0.0)
                        h2 = houts[m]
                        nc.vector.tensor_scalar_max(
                            h2[:, (b % 2) * t:(b % 2 + 1) * t], s[:], 0.0)
                        if b % 2 == 1:
                            dst = out_d[b - 1:b + 1, m * P:(m + 1) * P, :]
                            nc.sync.dma_start(
                                out=dst.rearrange("b p t -> p b t"),
                                in_=h2[:].rearrange("p (b t) -> p b t", b=2))
                prev = htiles
        # Tail pre-drain: the auto kernel-tail drain on SP must observe
        # every DMA queue and engine tick; feed SP one dependency per
        # pre-drain (same-proc waits merge) so the auto drain ends at zero.
        tail_deps = [i for i in nc.inst_map.values()
                     if type(i).__name__ == "InstDMACopy"]
        snap = list(nc.inst_map.values())
        for eng in ("DVE", "Activation"):
            last_e = [i for i in snap
                      if str(getattr(i, "engine", "")).endswith(eng)]
            if last_e:
                tail_deps.append(last_e[-1])
        tail_deps += [last_mm.ins, scan1.ins]
        for depi in tail_deps:
            dr = nc.sync.drain(fusable=False)
            bass._add_dep_helper(dr.ins, depi, sync=True,
                                 reason="tail pre-drain absorber")
    _assert_wait_budget(nc)
    return nc


# Instruction families exempt from the 1-sync-wait TPB events header (DMA
# descriptors and drains use the queue sync machinery). Everything that runs
# on a TPB engine sequencer (PE/DVE/ACT/Pool alike) is capacity-1.
_MULTI_WAIT_OK = {"InstDrain",
                  "InstEventSemaphore", "InstUnconditionalBranch",
                  "InstRegisterMove", "InstISA", "InstTensorLoad",
                  "InstTensorSave"}


def _assert_wait_budget(nc):
    bad = []
    for name, inst in nc.inst_map.items():
        ty = type(inst).__name__
        if ty in _MULTI_WAIT_OK:
            continue
        w = inst.sync_info.on_wait if inst.sync_info else []
        if len(w) > 1:
            bad.append((name, ty,
                        [f"{x.ant_name}>={x.wait_value}" for x in w]))
    if bad:
        raise RuntimeError(
            f"{len(bad)} instructions exceed the 1-sync-wait TPB limit, "
            f"first few: {bad[:5]}")


def _prep_core_inputs(Input, W0, Ws, bs, whs, core):
    """Host-side staging for one core: shard batch, transpose layer-0 input,
    pre-transpose weights into lhsT layout, broadcast recurrent weights."""
    bsl = slice(core * BLOC, (core + 1) * BLOC)
    return {
        "xT": np.ascontiguousarray(
            Input[bsl].transpose(0, 2, 1)).astype(np.float16),
        "w0t": np.ascontiguousarray(W0.T).astype(np.float16),
        "wst": np.ascontiguousarray(Ws.transpose(0, 2, 1)).astype(np.float16),
        "bias": np.ascontiguousarray(bs[:, None, :]).astype(np.float16),
        "wbc": np.ascontiguousarray(
            np.broadcast_to(whs.astype(np.float32)[:, :, None], (L, H, TCH))),
    }


def kernel(Input, W0, Ws, bs, whs):
    include_bias = bool(np.any(bs != 0))
    nc = build(include_bias=include_bias)
    in_maps = [_prep_core_inputs(Input, W0, Ws, bs, whs, r)
               for r in range(NCORES)]
    res = run_bass_kernel_spmd(nc, in_maps, core_ids=list(range(NCORES)))
    parts = [res.results[r]["out"] for r in range(NCORES)]  # [BLOC, H, T] each
    full = np.concatenate(parts, axis=0)  # [B, H, T]
    return np.ascontiguousarray(full.transpose(0, 2, 1)).astype(np.float32)



# revision 9
# speedup vs baseline: 1.3346x; 1.3346x over previous
"""Trainium2 Bass kernel for a 4-layer IndRNN (B=32, T=2048, I=256, H=512).

Math per layer: xp = x @ W.T (+b), then h_t = relu(xp_t + w (*) h_{t-1}),
with per-channel recurrent weight w = whs[l] in [0, 1).

The nonlinear scan decomposes into two linear-style DVE scans (see the
baseline derivation): dloc = linear scan of xp with factor w, q = min-scan,
h = relu(dloc - q). DVE scans cost ~2.1 ns/element regardless of dtype, so
this kernel additionally DECIMATES TIME BY 2: both scans run at length T/2
over pair-combined inputs, and the other parity is recovered on PE/ACT.

Per channel (K = T/2), validated exactly in fp64 (sim_check.py):
    xp_e[k] = xp[2k], xp_o[k] = xp[2k+1]
    y[k]        = w*xp_e[k] + xp_o[k]        (PE: W'=diag(w)W projection)
    dloc[k]     = w^2*dloc[k-1] + y[k]       (DVE scan 1, length T/2)
    u'[k]       = relu(xp_o[k]) - dloc[k]    (ACT relu + DVE subtract)
    M'[k+1]     = max(w^2*M'[k], u'[k])      (DVE scan 2;  M' = -M)
    h_odd[k]    = dloc[k] + M'[k+1]          (DVE add; exact, no relu)
    dloc_e[k]   = w*dloc[k-1] + xp_e[k]      (PE diag-matmul accumulate)
    h_even[k]   = relu(dloc_e[k] + w*M'[k])  (PE diag accumulate + ACT relu)

Sharding: data-parallel over batch, 4 batches per core, weights replicated.
Layout on device: [H(partitions), T/2(free)] per parity per batch; the host
pre-splits time parities and pre-transposes, and re-interleaves on the way
out, so the device never pays for transposes or strided DMA.
"""

import numpy as np
from contextlib import ExitStack

import concourse.bass as bass
import concourse.tile as tile
from concourse import mybir
from concourse.bass_utils import run_bass_kernel_spmd

dt = mybir.dt
Alu = mybir.AluOpType
Act = mybir.ActivationFunctionType

B, T, I, H, L = 32, 2048, 256, 512, 4
NCORES = 8
BLOC = B // NCORES
P = 128
TH = T // 2          # decimated scan length
FC = 512             # matmul free-dim chunk (= one PSUM bank of fp32)


def build(bloc=BLOC, t=T, include_bias=False, trace_sim=False):
    """Build the per-core Bass program (SPMD; identical on all cores)."""
    assert not include_bias, "bias path not implemented (bs==0 in this problem)"
    th = t // 2
    nf = th // FC
    ki, kh, m4 = I // P, H // P, H // P

    nc = bass.Bass("TRN2", target_bir_lowering=False, debug=False,
                   num_devices=NCORES)
    xe_d = nc.dram_tensor("xe", [bloc, I, th], dt.float16, kind="ExternalInput").ap()
    xo_d = nc.dram_tensor("xo", [bloc, I, th], dt.float16, kind="ExternalInput").ap()
    # plain lhsT weights and diag(w)-scaled lhsT weights, layer 0 and 1..3
    w0t_d = nc.dram_tensor("w0t", [I, H], dt.float16, kind="ExternalInput").ap()
    w0p_d = nc.dram_tensor("w0p", [I, H], dt.float16, kind="ExternalInput").ap()
    wst_d = nc.dram_tensor("wst", [L - 1, H, H], dt.float16, kind="ExternalInput").ap()
    wsp_d = nc.dram_tensor("wsp", [L - 1, H, H], dt.float16, kind="ExternalInput").ap()
    # diag(w) [L, m4, 128, 128] and w^2 vectors [L, H, 1]
    dg_d = nc.dram_tensor("dg", [L, m4, P, P], dt.float16, kind="ExternalInput").ap()
    w2_d = nc.dram_tensor("w2", [L, H, 1], dt.float32, kind="ExternalInput").ap()
    # output, parity-split: [b, H, parity, T/2]
    out_d = nc.dram_tensor("out", [bloc, H, 2, th], dt.float16,
                           kind="ExternalOutput").ap()

    with tile.TileContext(nc, trace_sim=trace_sim) as tc, ExitStack() as ctx:
        wpool = ctx.enter_context(tc.tile_pool(name="weights", bufs=1))
        xpool = ctx.enter_context(tc.tile_pool(name="xin", bufs=ki * bloc))
        hepool = ctx.enter_context(tc.tile_pool(name="he", bufs=20))
        hopool = ctx.enter_context(tc.tile_pool(name="ho", bufs=20))
        rpool = ctx.enter_context(tc.tile_pool(name="r", bufs=2))
        upool = ctx.enter_context(tc.tile_pool(name="u", bufs=2))
        dpool = ctx.enter_context(tc.tile_pool(name="dloc", bufs=2))
        mpool = ctx.enter_context(tc.tile_pool(name="mmin", bufs=2))
        spool = ctx.enter_context(tc.tile_pool(name="stage", bufs=2))
        psum = ctx.enter_context(tc.tile_pool(name="psum", bufs=2, space="PSUM"))

        # ---- persistent weights ----
        wt, wp = [], []   # wt[l][k] / wp[l][k] -> [128, H] fp16 lhsT
        for l in range(L):
            kprev = ki if l == 0 else kh
            tw, tp = [], []
            for k in range(kprev):
                w = wpool.tile([P, H], dt.float16, tag=f"w{l}{k}")
                p = wpool.tile([P, H], dt.float16, tag=f"p{l}{k}")
                if l == 0:
                    nc.gpsimd.dma_start(out=w[:], in_=w0t_d[k * P:(k + 1) * P, :])
                    nc.gpsimd.dma_start(out=p[:], in_=w0p_d[k * P:(k + 1) * P, :])
                else:
                    nc.gpsimd.dma_start(out=w[:], in_=wst_d[l - 1, k * P:(k + 1) * P, :])
                    nc.gpsimd.dma_start(out=p[:], in_=wsp_d[l - 1, k * P:(k + 1) * P, :])
                tw.append(w)
                tp.append(p)
            wt.append(tw)
            wp.append(tp)
        dg, w2 = [], []
        for l in range(L):
            td, tv = [], []
            for m in range(m4):
                d = wpool.tile([P, P], dt.float16, tag=f"dg{l}{m}")
                v = wpool.tile([P, 1], dt.float32, tag=f"w2{l}{m}")
                nc.gpsimd.dma_start(out=d[:], in_=dg_d[l, m, :, :])
                nc.gpsimd.dma_start(out=v[:], in_=w2_d[l, m * P:(m + 1) * P, :])
                td.append(d)
                tv.append(v)
            dg.append(td)
            w2.append(tv)

        # layer-0 inputs, all batches up front
        xe0, xo0 = [], []
        for b in range(bloc):
            te, to = [], []
            for k in range(ki):
                e = xpool.tile([P, th], dt.float16, tag="xe")
                o = xpool.tile([P, th], dt.float16, tag="xo")
                nc.gpsimd.dma_start(out=e[:], in_=xe_d[b, k * P:(k + 1) * P, :])
                nc.gpsimd.dma_start(out=o[:], in_=xo_d[b, k * P:(k + 1) * P, :])
                te.append(e)
                to.append(o)
            xe0.append(te)
            xo0.append(to)

        # Non-PE instructions carry only ONE sync-wait through walrus codegen.
        # Same-engine waits merge, so each engine first "claims" every
        # DMA-loaded operand it will read, leaving real ops a single wait.
        scratch = wpool.tile([P, L * m4], dt.float32, tag="scratch")
        scr_act = wpool.tile([P, L * m4], dt.float32, tag="scr_act")
        for l in range(L):
            for m in range(m4):
                col = slice(l * m4 + m, l * m4 + m + 1)
                nc.vector.tensor_copy(scratch[:, col], w2[l][m][:, 0:1])
                nc.scalar.activation(scr_act[:, col], w2[l][m][:, 0:1], Act.Relu)
        # rotating ACT-claimer scratch (fixed column would WAW itself)
        scr_rot = wpool.tile([P, 4 * bloc * L * m4], dt.float32, tag="scr_rot")
        # PE preamble: junk ldweights per weight tile so later real matmuls
        # never carry a weight-DMA wait.
        for l in range(L):
            for k in range(len(wt[l])):
                nc.tensor.ldweights(weights=wt[l][k][:, 0:P])
                nc.tensor.ldweights(weights=wp[l][k][:, 0:P])
            for m in range(m4):
                nc.tensor.ldweights(weights=dg[l][m][:])

        # ---- main loop: layer-outer, batch-inner for cross-unit pipelining;
        # the last layer runs m-outer so its 2 staging tiles double-buffer.
        he = {}       # (b, l, m) -> h_even tile
        ho = {}       # (b, l, m) -> h_odd tile
        he_ins = {}   # (b, l) -> last ACT h_even instruction
        ho_ins = {}   # (b, l) -> last DVE h_odd instruction
        stages = {}
        state = {"psum_cnt": 0, "ci": 0, "last_mm": None, "scan1": None,
                 "last_dve": None, "last_act": None}
        psum_readers = {}  # psum/r slot -> last instruction that read it

        def dve(ins):
            # pin DVE queue order so same-engine deps never need sem waits
            if state["last_dve"] is not None:
                bass._add_dep_helper(ins.ins, state["last_dve"].ins,
                                     sync=False, reason="DVE program order")
            state["last_dve"] = ins
            return ins

        def act(ins):
            if state["last_act"] is not None:
                bass._add_dep_helper(ins.ins, state["last_act"].ins,
                                     sync=False, reason="ACT program order")
            state["last_act"] = ins
            return ins

        def unit(l, b, m):
            kprev = ki if l == 0 else kh
            if l == 0:
                rhs_e, rhs_o = xe0[b], xo0[b]
            else:
                rhs_e = [he[(b, l - 1, k)] for k in range(kh)]
                rhs_o = [ho[(b, l - 1, k)] for k in range(kh)]
            ms = slice(m * P, (m + 1) * P)
            # two psum tiles; slot ids for WAR claimer bookkeeping
            py = psum.tile([P, th], dt.float32, tag="py")
            pe_ = psum.tile([P, th], dt.float32, tag="pe")
            sy = state["psum_cnt"] % 2
            se = 2 + sy
            state["psum_cnt"] += 1
            claimers = []
            for slot in (sy, se):
                old = psum_readers.get(slot)
                if old is not None:
                    ldw = nc.tensor.ldweights(weights=wt[l][0][:, 0:P])
                    bass._add_dep_helper(
                        ldw.ins, old.ins, sync=True,
                        reason="PE claimer for PSUM slot WAR")
                    claimers.append(ldw)
            if (m == 0 and l < L - 1) or (b == 0 and l == L - 1) or \
                    (l == L - 1 and m == 0):
                # absorb rhs producer ticks (DMA for l0; DVE h_odd and ACT
                # h_even of (b, l-1) otherwise)
                if l == 0:
                    for xt in (*rhs_e, *rhs_o):
                        claimers.append(nc.tensor.ldweights(
                            weights=xt[:, 0:P]))
                else:
                    for dep in (ho_ins[(b, l - 1)], he_ins[(b, l - 1)]):
                        ldw = nc.tensor.ldweights(weights=wt[l][0][:, 0:P])
                        bass._add_dep_helper(
                            ldw.ins, dep.ins, sync=True,
                            reason="PE claimer for rhs producers")
                        claimers.append(ldw)
            # PE block A: psum_y = W.x_o (xp_odd), psum_e = W.x_e
            first = True
            last_g1 = None
            for f in range(nf):
                fs = slice(f * FC, (f + 1) * FC)
                for k in range(kprev):
                    mm = nc.tensor.matmul(
                        py[:, fs], lhsT=wt[l][k][:, ms], rhs=rhs_o[k][:, fs],
                        start=(k == 0), stop=(k == kprev - 1))
                    if first:
                        for cl in claimers:
                            bass._add_dep_helper(
                                mm.ins, cl.ins, sync=False,
                                reason="order claimers before MMs")
                        first = False
                    last_g1 = mm
            for f in range(nf):
                fs = slice(f * FC, (f + 1) * FC)
                for k in range(kprev):
                    nc.tensor.matmul(
                        pe_[:, fs], lhsT=wt[l][k][:, ms], rhs=rhs_e[k][:, fs],
                        start=(k == 0), stop=(k == kprev - 1))
            # ACT r = relu(xp_odd) (reads psum_y mid-chain)
            r = rpool.tile([P, th], dt.float16, tag="r")
            ci = state["ci"]
            c0 = act(nc.scalar.activation(scr_rot[:, ci:ci + 1],
                                          w2[l][m][:, 0:1], Act.Relu))
            state["ci"] = ci + 1
            old_r = psum_readers.get(4)  # previous u instruction
            if old_r is not None:
                bass._add_dep_helper(
                    c0.ins, old_r.ins, sync=True,
                    reason="ACT claimer for r-tile WAR")
            if l == L - 1:
                # no h_even ACT op on the last layer, so absorb the PE
                # group-1 tick in an extra claimer to keep r at one wait
                ci = state["ci"]
                c1 = act(nc.scalar.activation(scr_rot[:, ci:ci + 1],
                                              w2[l][m][:, 0:1], Act.Relu))
                state["ci"] = ci + 1
                bass._add_dep_helper(c1.ins, last_g1.ins, sync=True,
                                     reason="ACT claimer: PE group1 tick")
            r_ins = act(nc.scalar.activation(r[:], py[:], Act.Relu))
            # PE block B: psum_y += W'.x_e  -> y
            ldw = nc.tensor.ldweights(weights=wp[l][0][:, 0:P])
            bass._add_dep_helper(
                ldw.ins, r_ins.ins, sync=True,
                reason="PE claimer: y-accum waits ACT r read")
            first = True
            for f in range(nf):
                fs = slice(f * FC, (f + 1) * FC)
                for k in range(kprev):
                    mm = nc.tensor.matmul(
                        py[:, fs], lhsT=wp[l][k][:, ms], rhs=rhs_e[k][:, fs],
                        start=False, stop=(k == kprev - 1),
                        skip_group_check=True)
                    if first:
                        bass._add_dep_helper(
                            mm.ins, ldw.ins, sync=False,
                            reason="order claimer before MMs")
                        first = False
                    state["last_mm"] = mm
            # DVE scan 1: dloc (odd positions); out at col 2 so later fp16
            # tensor_tensor reads are 4B-aligned (2x mode)
            dloc = dpool.tile([P, th + 2], dt.float16, tag="dloc")
            ms0 = dve(nc.vector.memset(dloc[:, 1:2], 0.0))
            # absorb the ACT r tick here so scan1 keeps only its PE wait
            bass._add_dep_helper(ms0.ins, r_ins.ins, sync=True,
                                 reason="DVE claimer: ACT r tick")
            wv = w2[l][m][:, 0:1].broadcast_to((P, th))
            scan1 = dve(nc.vector.tensor_tensor_scan(
                out=dloc[:, 2:th + 2], data0=wv, data1=py[:],
                initial=0.0, op0=Alu.mult, op1=Alu.add))
            state["scan1"] = scan1
            psum_readers[sy] = scan1
            # DVE u' = r - dloc (feeds the max-scan for M' = -M)
            u = upool.tile([P, th], dt.float16, tag="u")
            u_ins = dve(nc.vector.tensor_tensor(
                out=u[:], in0=r[:], in1=dloc[:, 2:th + 2], op=Alu.subtract))
            psum_readers[4] = u_ins
            # DVE scan 2: M'[k+1] = max(w^2 M'[k], u'[k])
            mt = mpool.tile([P, th + 2], dt.float16, tag="mmin")
            dve(nc.vector.memset(mt[:, 1:2], 0.0))
            dve(nc.vector.tensor_tensor_scan(
                out=mt[:, 2:th + 2], data0=wv, data1=u[:],
                initial=0.0, op0=Alu.mult, op1=Alu.max))
            # PE block C: psum_e += diag(w).dloc_shift + diag(w).M'_shift
            for f in range(nf):
                fs = slice(f * FC, (f + 1) * FC)
                nc.tensor.matmul(
                    pe_[:, fs], lhsT=dg[l][m][:],
                    rhs=dloc[:, 1 + f * FC:1 + f * FC + FC],
                    start=False, stop=False, skip_group_check=True)
                state["last_mm"] = nc.tensor.matmul(
                    pe_[:, fs], lhsT=dg[l][m][:],
                    rhs=mt[:, 1 + f * FC:1 + f * FC + FC],
                    start=False, stop=True, skip_group_check=True)
            if l < L - 1:
                # h_odd on DVE, h_even on ACT
                h_o = hopool.tile([P, th], dt.float16, tag="ho")
                ho_ins[(b, l)] = dve(nc.vector.tensor_tensor(
                    out=h_o[:], in0=dloc[:, 2:th + 2], in1=mt[:, 2:th + 2],
                    op=Alu.add))
                ho[(b, l, m)] = h_o
                h_e = hepool.tile([P, th], dt.float16, tag="he")
                he_ins[(b, l)] = act(nc.scalar.activation(h_e[:], pe_[:],
                                                          Act.Relu))
                he[(b, l, m)] = h_e
                psum_readers[se] = he_ins[(b, l)]
            else:
                # final layer: both halves written by DVE into the batch-pair
                # staging tile, one DMA per (b-pair, m)
                if b % 2 == 0:
                    st = spool.tile([P, 2 * t], dt.float16, tag="stage")
                    stages[m] = st
                    # first toucher claims the store-DMA WAR tick
                    dve(nc.vector.memset(st[:, 0:1], 0.0))
                st = stages[m]
                off = (b % 2) * t
                # DVE claimer: absorb the PE block-C tick so ev keeps one wait
                cs = state["ci"]
                cdve = dve(nc.vector.memset(
                    scratch[:, (cs % (L * m4)):(cs % (L * m4)) + 1], 0.0))
                bass._add_dep_helper(cdve.ins, state["last_mm"].ins, sync=True,
                                     reason="DVE claimer: PE blockC tick")
                ev = dve(nc.vector.tensor_scalar_max(
                    st[:, off:off + th], pe_[:], 0.0))
                psum_readers[se] = ev
                dve(nc.vector.tensor_tensor(
                    out=st[:, off + th:off + t],
                    in0=dloc[:, 2:th + 2], in1=mt[:, 2:th + 2], op=Alu.add))
                if b % 2 == 1:
                    dst = out_d[b - 1:b + 1, ms, :, :]
                    nc.sync.dma_start(
                        out=dst.rearrange("b p r t -> p b r t"),
                        in_=st[:].rearrange("p (b r t) -> p b r t", b=2, r=2))

        for l in range(L - 1):
            for b in range(bloc):
                for m in range(m4):
                    unit(l, b, m)
        for m in range(m4):
            for b in range(bloc):
                unit(L - 1, b, m)

        # ---- tail pre-drain (see baseline): absorb every DMA queue and
        # engine tick so the auto kernel-tail drain ends at zero waits.
        tail_deps = [i for i in nc.inst_map.values()
                     if type(i).__name__ == "InstDMACopy"]
        snap = list(nc.inst_map.values())
        compute_tys = {"InstTensorScalarPtr", "InstTensorTensor",
                       "InstActivation", "InstTensorCopy", "InstMemset"}
        for eng in ("DVE", "Activation"):
            last_e = [i for i in snap
                      if str(getattr(i, "engine", "")).endswith(eng)
                      and type(i).__name__ in compute_tys]
            if last_e:
                tail_deps.append(last_e[-1])
        tail_deps += [state["last_mm"].ins, state["scan1"].ins]
        for depi in tail_deps:
            dr = nc.sync.drain(fusable=False)
            bass._add_dep_helper(dr.ins, depi, sync=True,
                                 reason="tail pre-drain absorber")
    _assert_wait_budget(nc)
    return nc


_MULTI_WAIT_OK = {"InstDrain",
                  "InstEventSemaphore", "InstUnconditionalBranch",
                  "InstRegisterMove", "InstISA", "InstTensorLoad",
                  "InstTensorSave"}


def _assert_wait_budget(nc):
    bad = []
    for name, inst in nc.inst_map.items():
        ty = type(inst).__name__
        if ty in _MULTI_WAIT_OK:
            continue
        w = inst.sync_info.on_wait if inst.sync_info else []
        if len(w) > 1:
            bad.append((name, ty,
                        [f"{x.ant_name}>={x.wait_value}" for x in w]))
    if bad:
        raise RuntimeError(
            f"{len(bad)} instructions exceed the 1-sync-wait TPB limit, "
            f"first few: {bad[:5]}")


def _prep_core_inputs(Input, W0, Ws, bs, whs, core):
    """Host-side staging for one core: shard batch, transpose + parity-split
    the layer-0 input, build plain and diag(w)-scaled lhsT weights, diag
    matrices, and w^2 scan vectors."""
    bsl = slice(core * BLOC, (core + 1) * BLOC)
    xT = Input[bsl].transpose(0, 2, 1).astype(np.float16)  # [bloc, I, T]
    w0t = W0.T.astype(np.float16)                          # [I, H]
    wst = Ws.transpose(0, 2, 1).astype(np.float16)         # [L-1, H, H]
    whsf = whs.astype(np.float32)                          # [L, H]
    w0p = (W0.T * whsf[0][None, :]).astype(np.float16)
    wsp = (Ws.transpose(0, 2, 1) * whsf[1:, None, :]).astype(np.float16)
    m4 = H // P
    dgm = np.zeros((L, m4, P, P), np.float16)
    for l in range(L):
        for m in range(m4):
            blk = whsf[l, m * P:(m + 1) * P]
            np.fill_diagonal(dgm[l, m], blk.astype(np.float16))
    return {
        "xe": np.ascontiguousarray(xT[:, :, 0::2]),
        "xo": np.ascontiguousarray(xT[:, :, 1::2]),
        "w0t": np.ascontiguousarray(w0t),
        "w0p": np.ascontiguousarray(w0p),
        "wst": np.ascontiguousarray(wst),
        "wsp": np.ascontiguousarray(wsp),
        "dg": dgm,
        "w2": np.ascontiguousarray((whsf * whsf)[:, :, None]),
    }


def kernel(Input, W0, Ws, bs, whs):
    include_bias = bool(np.any(bs != 0))
    nc = build(include_bias=include_bias)
    in_maps = [_prep_core_inputs(Input, W0, Ws, bs, whs, r)
               for r in range(NCORES)]
    res = run_bass_kernel_spmd(nc, in_maps, core_ids=list(range(NCORES)))
    parts = [res.results[r]["out"] for r in range(NCORES)]  # [BLOC, H, 2, T/2]
    po = np.concatenate(parts, axis=0)  # [B, H, 2, T/2]
    full = np.empty((B, H, T), np.float16)
    full[:, :, 0::2] = po[:, :, 0, :]
    full[:, :, 1::2] = po[:, :, 1, :]
    return np.ascontiguousarray(full.transpose(0, 2, 1)).astype(np.float32)


# revision 15
# speedup vs baseline: 1.9499x; 1.4610x over previous
"""Trainium2 Bass kernel for a 4-layer IndRNN (B=32, T=2048, I=256, H=512).

Math per layer: xp = x @ W.T (+b), then h_t = relu(xp_t + w (*) h_{t-1}),
with per-channel recurrent weight w = whs[l] in [0, 1).

The nonlinear scan decomposes into two linear-style DVE scans (see the
baseline derivation): dloc = linear scan of xp with factor w, q = min-scan,
h = relu(dloc - q). DVE scans cost ~2.1 ns/element regardless of dtype, so
this kernel additionally DECIMATES TIME BY 2: both scans run at length T/2
over pair-combined inputs, and the other parity is recovered on PE/ACT.

Per channel (K = T/2), validated exactly in fp64 (sim_check.py):
    xp_e[k] = xp[2k], xp_o[k] = xp[2k+1]
    y[k]        = w*xp_e[k] + xp_o[k]        (PE: W'=diag(w)W projection)
    dloc[k]     = w^2*dloc[k-1] + y[k]       (DVE scan 1, length T/2)
    u'[k]       = relu(xp_o[k]) - dloc[k]    (ACT relu + DVE subtract)
    M'[k+1]     = max(w^2*M'[k], u'[k])      (DVE scan 2;  M' = -M)
    h_odd[k]    = dloc[k] + M'[k+1]          (DVE add; exact, no relu)
    dloc_e[k]   = w*dloc[k-1] + xp_e[k]      (PE diag-matmul accumulate)
    h_even[k]   = relu(dloc_e[k] + w*M'[k])  (PE diag accumulate + ACT relu)

Sharding: data-parallel over batch, 4 batches per core, weights replicated.
Layout on device: [H(partitions), T/2(free)] per parity per batch; the host
pre-splits time parities and pre-transposes, and re-interleaves on the way
out, so the device never pays for transposes or strided DMA.
"""

import numpy as np
from contextlib import ExitStack

import concourse.bass as bass
import concourse.tile as tile
from concourse import mybir
from concourse.bass_utils import run_bass_kernel_spmd

dt = mybir.dt
Alu = mybir.AluOpType
Act = mybir.ActivationFunctionType

B, T, I, H, L = 32, 2048, 256, 512, 4
NCORES = 8
BLOC = B // NCORES
P = 128
TH = T // 2          # decimated scan length
FC = 512             # matmul free-dim chunk (= one PSUM bank of fp32)


def build(bloc=BLOC, t=T, include_bias=False, trace_sim=False):
    """Build the per-core Bass program (SPMD; identical on all cores)."""
    assert not include_bias, "bias path not implemented (bs==0 in this problem)"
    th = t // 2
    nf = th // FC
    ki, kh, m4 = I // P, H // P, H // P

    nc = bass.Bass("TRN2", target_bir_lowering=False, debug=False,
                   num_devices=NCORES)
    xe_d = nc.dram_tensor("xe", [bloc, I, th], dt.float16, kind="ExternalInput").ap()
    xo_d = nc.dram_tensor("xo", [bloc, I, th], dt.float16, kind="ExternalInput").ap()
    # plain lhsT weights and diag(w)-scaled lhsT weights, layer 0 and 1..3
    w0t_d = nc.dram_tensor("w0t", [I, H], dt.float16, kind="ExternalInput").ap()
    wst_d = nc.dram_tensor("wst", [L - 1, H, H], dt.float16, kind="ExternalInput").ap()
    idm_d = nc.dram_tensor("idm", [P, P], dt.float16, kind="ExternalInput").ap()
    wv1_d = nc.dram_tensor("wv1", [L, H, 1], dt.float32, kind="ExternalInput").ap()
    # diag(w) [L, m4, 128, 128] and w^2 vectors [L, H, 1]
    dg_d = nc.dram_tensor("dg", [L, m4, P, P], dt.float16, kind="ExternalInput").ap()
    w2_d = nc.dram_tensor("w2", [L, H, 1], dt.float32, kind="ExternalInput").ap()
    # output, parity-split: [b, H, parity, T/2]
    out_d = nc.dram_tensor("out", [bloc, H, 2, th], dt.float16,
                           kind="ExternalOutput").ap()

    with tile.TileContext(nc, trace_sim=trace_sim) as tc, ExitStack() as ctx:
        wpool = ctx.enter_context(tc.tile_pool(name="weights", bufs=1))
        xpool = ctx.enter_context(tc.tile_pool(name="xin", bufs=ki * bloc))
        hepool = ctx.enter_context(tc.tile_pool(name="he", bufs=20))
        hopool = ctx.enter_context(tc.tile_pool(name="ho", bufs=20))
        rpool = ctx.enter_context(tc.tile_pool(name="r", bufs=2))
        tpool = ctx.enter_context(tc.tile_pool(name="t", bufs=3))
        upool = ctx.enter_context(tc.tile_pool(name="u", bufs=2))
        dpool = ctx.enter_context(tc.tile_pool(name="dloc", bufs=2))
        mpool = ctx.enter_context(tc.tile_pool(name="mmin", bufs=2))
        spool = ctx.enter_context(tc.tile_pool(name="stage", bufs=2))
        psum = ctx.enter_context(tc.tile_pool(name="psum", bufs=2, space="PSUM"))

        # ---- persistent weights ----
        wt = []   # wt[l][k] -> [128, H] fp16 lhsT
        for l in range(L):
            kprev = ki if l == 0 else kh
            tw = []
            for k in range(kprev):
                w = wpool.tile([P, H], dt.float16, tag=f"w{l}{k}")
                if l == 0:
                    nc.gpsimd.dma_start(out=w[:], in_=w0t_d[k * P:(k + 1) * P, :])
                else:
                    nc.gpsimd.dma_start(out=w[:], in_=wst_d[l - 1, k * P:(k + 1) * P, :])
                tw.append(w)
            wt.append(tw)
        idm = wpool.tile([P, P], dt.float16, tag="idm")
        nc.gpsimd.dma_start(out=idm[:], in_=idm_d)
        dg, w2, w1 = [], [], []
        for l in range(L):
            td, tv, tv1 = [], [], []
            for m in range(m4):
                d = wpool.tile([P, P], dt.float16, tag=f"dg{l}{m}")
                v = wpool.tile([P, 1], dt.float32, tag=f"w2{l}{m}")
                v1 = wpool.tile([P, 1], dt.float32, tag=f"w1{l}{m}")
                nc.gpsimd.dma_start(out=d[:], in_=dg_d[l, m, :, :])
                nc.gpsimd.dma_start(out=v[:], in_=w2_d[l, m * P:(m + 1) * P, :])
                nc.gpsimd.dma_start(out=v1[:], in_=wv1_d[l, m * P:(m + 1) * P, :])
                td.append(d)
                tv.append(v)
                tv1.append(v1)
            dg.append(td)
            w2.append(tv)
            w1.append(tv1)

        # layer-0 inputs, all batches up front
        xe0, xo0 = [], []
        for b in range(bloc):
            te, to = [], []
            for k in range(ki):
                e = xpool.tile([P, th], dt.float16, tag="xe")
                o = xpool.tile([P, th], dt.float16, tag="xo")
                nc.gpsimd.dma_start(out=e[:], in_=xe_d[b, k * P:(k + 1) * P, :])
                nc.gpsimd.dma_start(out=o[:], in_=xo_d[b, k * P:(k + 1) * P, :])
                te.append(e)
                to.append(o)
            xe0.append(te)
            xo0.append(to)

        # Non-PE instructions carry only ONE sync-wait through walrus codegen.
        # Same-engine waits merge, so each engine first "claims" every
        # DMA-loaded operand it will read, leaving real ops a single wait.
        scratch = wpool.tile([P, L * m4], dt.float32, tag="scratch")
        scr_act = wpool.tile([P, 2 * L * m4], dt.float32, tag="scr_act")
        for l in range(L):
            for m in range(m4):
                col = slice(l * m4 + m, l * m4 + m + 1)
                col2 = slice(L * m4 + l * m4 + m, L * m4 + l * m4 + m + 1)
                nc.vector.tensor_copy(scratch[:, col], w2[l][m][:, 0:1])
                nc.scalar.activation(scr_act[:, col], w2[l][m][:, 0:1], Act.Relu)
                nc.scalar.activation(scr_act[:, col2], w1[l][m][:, 0:1], Act.Relu)
        # rotating ACT-claimer scratch (fixed column would WAW itself)
        scr_rot = wpool.tile([P, 4 * bloc * L * m4], dt.float32, tag="scr_rot")
        # PE preamble: junk ldweights per weight tile so later real matmuls
        # never carry a weight-DMA wait.
        for l in range(L):
            for k in range(len(wt[l])):
                nc.tensor.ldweights(weights=wt[l][k][:, 0:P])
            for m in range(m4):
                nc.tensor.ldweights(weights=dg[l][m][:])
        nc.tensor.ldweights(weights=idm[:])

        # ---- main loop: layer-outer, batch-inner for cross-unit pipelining;
        # the last layer runs m-outer so its 2 staging tiles double-buffer.
        he = {}       # (b, l, m) -> h_even tile
        ho = {}       # (b, l, m) -> h_odd tile
        he_ins = {}   # (b, l) -> last ACT h_even instruction
        ho_ins = {}   # (b, l) -> last DVE h_odd instruction
        stages = {}
        state = {"psum_cnt": 0, "ci": 0, "last_mm": None, "scan1": None,
                 "last_dve": None, "last_act": None, "tail": None}
        psum_readers = {}  # psum/r slot -> last instruction that read it

        def dve(ins):
            # pin DVE queue order so same-engine deps never need sem waits
            if state["last_dve"] is not None:
                bass._add_dep_helper(ins.ins, state["last_dve"].ins,
                                     sync=False, reason="DVE program order")
            state["last_dve"] = ins
            return ins

        def act(ins):
            if state["last_act"] is not None:
                bass._add_dep_helper(ins.ins, state["last_act"].ins,
                                     sync=False, reason="ACT program order")
            state["last_act"] = ins
            return ins

        def unit(l, b, m):
            kprev = ki if l == 0 else kh
            if l == 0:
                rhs_e, rhs_o = xe0[b], xo0[b]
            else:
                rhs_e = [he[(b, l - 1, k)] for k in range(kh)]
                rhs_o = [ho[(b, l - 1, k)] for k in range(kh)]
            ms = slice(m * P, (m + 1) * P)
            # two psum tiles; slot ids for WAR claimer bookkeeping
            py = psum.tile([P, th], dt.float32, tag="py")
            pe_ = psum.tile([P, th], dt.float32, tag="pe")
            sy = state["psum_cnt"] % 2
            se = 2 + sy
            state["psum_cnt"] += 1
            claimers = []
            for slot in (sy, se):
                old = psum_readers.get(slot)
                if old is not None:
                    ldw = nc.tensor.ldweights(weights=wt[l][0][:, 0:P])
                    bass._add_dep_helper(
                        ldw.ins, old.ins, sync=True,
                        reason="PE claimer for PSUM slot WAR")
                    claimers.append(ldw)
            if (m == 0 and l < L - 1) or (b == 0 and l == L - 1) or \
                    (l == L - 1 and m == 0):
                # absorb rhs producer ticks (DMA for l0; DVE h_odd and ACT
                # h_even of (b, l-1) otherwise)
                if l == 0:
                    for xt in (*rhs_e, *rhs_o):
                        claimers.append(nc.tensor.ldweights(
                            weights=xt[:, 0:P]))
                else:
                    for dep in (ho_ins[(b, l - 1)], he_ins[(b, l - 1)]):
                        ldw = nc.tensor.ldweights(weights=wt[l][0][:, 0:P])
                        bass._add_dep_helper(
                            ldw.ins, dep.ins, sync=True,
                            reason="PE claimer for rhs producers")
                        claimers.append(ldw)
            # PE block A: psum_y = W.x_o (xp_odd), psum_e = W.x_e
            first = True
            last_g1 = None
            for f in range(nf):
                fs = slice(f * FC, (f + 1) * FC)
                for k in range(kprev):
                    mm = nc.tensor.matmul(
                        py[:, fs], lhsT=wt[l][k][:, ms], rhs=rhs_o[k][:, fs],
                        start=(k == 0), stop=(k == kprev - 1))
                    if first:
                        for cl in claimers:
                            bass._add_dep_helper(
                                mm.ins, cl.ins, sync=False,
                                reason="order claimers before MMs")
                        first = False
                    last_g1 = mm
            for f in range(nf):
                fs = slice(f * FC, (f + 1) * FC)
                for k in range(kprev):
                    nc.tensor.matmul(
                        pe_[:, fs], lhsT=wt[l][k][:, ms], rhs=rhs_e[k][:, fs],
                        start=(k == 0), stop=(k == kprev - 1))
            # ACT r = relu(xp_odd) (reads psum_y mid-chain)
            r = rpool.tile([P, th], dt.float16, tag="r")
            ci = state["ci"]
            c0 = act(nc.scalar.activation(scr_rot[:, ci:ci + 1],
                                          w2[l][m][:, 0:1], Act.Relu))
            state["ci"] = ci + 1
            old_r = psum_readers.get(4)  # previous u instruction
            if old_r is not None:
                bass._add_dep_helper(
                    c0.ins, old_r.ins, sync=True,
                    reason="ACT claimer for r-tile WAR")
            if l == L - 1:
                # no h_even ACT op on the last layer, so absorb the PE
                # group-1 tick in an extra claimer to keep r at one wait
                ci = state["ci"]
                c1 = act(nc.scalar.activation(scr_rot[:, ci:ci + 1],
                                              w2[l][m][:, 0:1], Act.Relu))
                state["ci"] = ci + 1
                bass._add_dep_helper(c1.ins, last_g1.ins, sync=True,
                                     reason="ACT claimer: PE group1 tick")
            r_ins = act(nc.scalar.activation(r[:], py[:], Act.Relu))
            # ACT t = w (*) xp_e (per-partition scale), SBUF fp16
            t_ = tpool.tile([P, th], dt.float16, tag="t")
            t_ins = act(nc.scalar.activation(t_[:], pe_[:], Act.Copy,
                                             scale=w1[l][m][:, 0:1]))
            # emit the previous unit's tail PE block here: it overlaps the
            # current unit's ACT stage and precedes block B in PE order
            if state["tail"] is not None:
                state["tail"][0]()
            # PE block B: psum_y += I.t  -> y  (2 matmuls, K=128)
            ldw = nc.tensor.ldweights(weights=idm[:])
            bass._add_dep_helper(
                ldw.ins, t_ins.ins, sync=True,
                reason="PE claimer: y-accum waits ACT r+t reads")
            first = True
            for f in range(nf):
                fs = slice(f * FC, (f + 1) * FC)
                mm = nc.tensor.matmul(
                    py[:, fs], lhsT=idm[:], rhs=t_[:, fs],
                    start=False, stop=True, skip_group_check=True)
                if first:
                    bass._add_dep_helper(
                        mm.ins, ldw.ins, sync=False,
                        reason="order claimer before MMs")
                    first = False
                state["last_mm"] = mm
            # previous unit's tail rest (ACT h_even / DVE h_odd / stores)
            if state["tail"] is not None:
                state["tail"][1]()
                state["tail"] = None
            # DVE scan 1: dloc (odd positions); out at col 2 so later fp16
            # tensor_tensor reads are 4B-aligned (2x mode)
            dloc = dpool.tile([P, th + 2], dt.float16, tag="dloc")
            ms0 = dve(nc.vector.memset(dloc[:, 1:2], 0.0))
            # absorb the ACT r tick here so scan1 keeps only its PE wait
            bass._add_dep_helper(ms0.ins, r_ins.ins, sync=True,
                                 reason="DVE claimer: ACT r tick")
            wv = w2[l][m][:, 0:1].broadcast_to((P, th))
            scan1 = dve(nc.vector.tensor_tensor_scan(
                out=dloc[:, 2:th + 2], data0=wv, data1=py[:],
                initial=0.0, op0=Alu.mult, op1=Alu.add))
            state["scan1"] = scan1
            psum_readers[sy] = scan1
            # DVE u' = r - dloc (feeds the max-scan for M' = -M)
            u = upool.tile([P, th], dt.float16, tag="u")
            u_ins = dve(nc.vector.tensor_tensor(
                out=u[:], in0=r[:], in1=dloc[:, 2:th + 2], op=Alu.subtract))
            psum_readers[4] = u_ins
            # DVE scan 2: M'[k+1] = max(w^2 M'[k], u'[k])
            mt = mpool.tile([P, th + 2], dt.float16, tag="mmin")
            dve(nc.vector.memset(mt[:, 1:2], 0.0))
            dve(nc.vector.tensor_tensor_scan(
                out=mt[:, 2:th + 2], data0=wv, data1=u[:],
                initial=0.0, op0=Alu.mult, op1=Alu.max))
            cbox = {}

            def tail_pe():
                # PE block C: psum_e += diag(w).dloc_shift + diag(w).M'_shift
                for f in range(nf):
                    fs = slice(f * FC, (f + 1) * FC)
                    nc.tensor.matmul(
                        pe_[:, fs], lhsT=dg[l][m][:],
                        rhs=dloc[:, 1 + f * FC:1 + f * FC + FC],
                        start=False, stop=False, skip_group_check=True)
                    cbox["c"] = nc.tensor.matmul(
                        pe_[:, fs], lhsT=dg[l][m][:],
                        rhs=mt[:, 1 + f * FC:1 + f * FC + FC],
                        start=False, stop=True, skip_group_check=True)
                    state["last_mm"] = cbox["c"]

            def tail_rest(last_c=None):
                if last_c is None:
                    last_c = state["last_mm"]
                if l < L - 1:
                    # h_odd on DVE, h_even on ACT
                    h_o = hopool.tile([P, th], dt.float16, tag="ho")
                    ho_ins[(b, l)] = dve(nc.vector.tensor_tensor(
                        out=h_o[:], in0=dloc[:, 2:th + 2],
                        in1=mt[:, 2:th + 2], op=Alu.add))
                    ho[(b, l, m)] = h_o
                    h_e = hepool.tile([P, th], dt.float16, tag="he")
                    # ACT claimer: absorb the PE block-C tick so h_even
                    # keeps only its own-engine ordering wait
                    ci2 = state["ci"]
                    c2 = act(nc.scalar.activation(
                        scr_rot[:, ci2:ci2 + 1], w2[l][m][:, 0:1], Act.Relu))
                    state["ci"] = ci2 + 1
                    bass._add_dep_helper(c2.ins, last_c.ins, sync=True,
                                         reason="ACT claimer: PE blockC tick")
                    he_ins[(b, l)] = act(nc.scalar.activation(
                        h_e[:], pe_[:], Act.Relu))
                    he[(b, l, m)] = h_e
                    psum_readers[se] = he_ins[(b, l)]
                else:
                    # final layer: both halves written by DVE into the
                    # batch-pair staging tile, one DMA per (b-pair, m)
                    if b % 2 == 0:
                        st = spool.tile([P, 2 * t], dt.float16, tag="stage")
                        stages[m] = st
                        # first toucher claims the store-DMA WAR tick
                        dve(nc.vector.memset(st[:, 0:1], 0.0))
                    st = stages[m]
                    off = (b % 2) * t
                    # DVE claimers: absorb the ACT t tick and the PE
                    # block-C tick so ev keeps a single wait
                    cs = state["ci"]
                    ca = dve(nc.vector.memset(
                        scratch[:, (cs % (L * m4)):(cs % (L * m4)) + 1], 0.0))
                    bass._add_dep_helper(ca.ins, t_ins.ins, sync=True,
                                         reason="DVE claimer: ACT t tick")
                    cs += 1
                    cdve = dve(nc.vector.memset(
                        scratch[:, (cs % (L * m4)):(cs % (L * m4)) + 1], 0.0))
                    bass._add_dep_helper(cdve.ins, last_c.ins, sync=True,
                                         reason="DVE claimer: PE blockC tick")
                    ev = dve(nc.vector.tensor_scalar_max(
                        st[:, off:off + th], pe_[:], 0.0))
                    psum_readers[se] = ev
                    dve(nc.vector.tensor_tensor(
                        out=st[:, off + th:off + t],
                        in0=dloc[:, 2:th + 2], in1=mt[:, 2:th + 2],
                        op=Alu.add))
                    if b % 2 == 1:
                        dst = out_d[b - 1:b + 1, ms, :, :]
                        nc.sync.dma_start(
                            out=dst.rearrange("b p r t -> p b r t"),
                            in_=st[:].rearrange("p (b r t) -> p b r t",
                                                b=2, r=2))
            state["tail"] = (tail_pe, lambda: tail_rest(last_c=cbox["c"]))

        for l in range(L - 1):
            for b in range(bloc):
                for m in range(m4):
                    unit(l, b, m)
        for m in range(m4):
            for b in range(bloc):
                unit(L - 1, b, m)
        state["tail"][0]()
        state["tail"][1]()
        state["tail"] = None

        # ---- tail pre-drain (see baseline): absorb every DMA queue and
        # engine tick so the auto kernel-tail drain ends at zero waits.
        tail_deps = [i for i in nc.inst_map.values()
                     if type(i).__name__ == "InstDMACopy"]
        snap = list(nc.inst_map.values())
        compute_tys = {"InstTensorScalarPtr", "InstTensorTensor",
                       "InstActivation", "InstTensorCopy", "InstMemset"}
        for eng in ("DVE", "Activation"):
            last_e = [i for i in snap
                      if str(getattr(i, "engine", "")).endswith(eng)
                      and type(i).__name__ in compute_tys]
            if last_e:
                tail_deps.append(last_e[-1])
        tail_deps += [state["last_mm"].ins, state["scan1"].ins]
        for depi in tail_deps:
            dr = nc.sync.drain(fusable=False)
            bass._add_dep_helper(dr.ins, depi, sync=True,
                                 reason="tail pre-drain absorber")
    _assert_wait_budget(nc)
    return nc


_MULTI_WAIT_OK = {"InstDrain",
                  "InstEventSemaphore", "InstUnconditionalBranch",
                  "InstRegisterMove", "InstISA", "InstTensorLoad",
                  "InstTensorSave"}


def _assert_wait_budget(nc):
    bad = []
    for name, inst in nc.inst_map.items():
        ty = type(inst).__name__
        if ty in _MULTI_WAIT_OK:
            continue
        w = inst.sync_info.on_wait if inst.sync_info else []
        if len(w) > 1:
            bad.append((name, ty,
                        [f"{x.ant_name}>={x.wait_value}" for x in w]))
    if bad:
        raise RuntimeError(
            f"{len(bad)} instructions exceed the 1-sync-wait TPB limit, "
            f"first few: {bad[:5]}")


def _prep_core_inputs(Input, W0, Ws, bs, whs, core):
    """Host-side staging for one core: shard batch, transpose + parity-split
    the layer-0 input, build plain and diag(w)-scaled lhsT weights, diag
    matrices, and w^2 scan vectors."""
    bsl = slice(core * BLOC, (core + 1) * BLOC)
    xT = Input[bsl].transpose(0, 2, 1).astype(np.float16)  # [bloc, I, T]
    w0t = W0.T.astype(np.float16)                          # [I, H]
    wst = Ws.transpose(0, 2, 1).astype(np.float16)         # [L-1, H, H]
    whsf = whs.astype(np.float32)                          # [L, H]
    m4 = H // P
    dgm = np.zeros((L, m4, P, P), np.float16)
    for l in range(L):
        for m in range(m4):
            blk = whsf[l, m * P:(m + 1) * P]
            np.fill_diagonal(dgm[l, m], blk.astype(np.float16))
    return {
        "xe": np.ascontiguousarray(xT[:, :, 0::2]),
        "xo": np.ascontiguousarray(xT[:, :, 1::2]),
        "w0t": np.ascontiguousarray(w0t),
        "wst": np.ascontiguousarray(wst),
        "idm": np.eye(P, dtype=np.float16),
        "dg": dgm,
        "w2": np.ascontiguousarray((whsf * whsf)[:, :, None]),
        "wv1": np.ascontiguousarray(whsf[:, :, None]),
    }


def kernel(Input, W0, Ws, bs, whs):
    include_bias = bool(np.any(bs != 0))
    nc = build(include_bias=include_bias)
    in_maps = [_prep_core_inputs(Input, W0, Ws, bs, whs, r)
               for r in range(NCORES)]
    res = run_bass_kernel_spmd(nc, in_maps, core_ids=list(range(NCORES)))
    parts = [res.results[r]["out"] for r in range(NCORES)]  # [BLOC, H, 2, T/2]
    po = np.concatenate(parts, axis=0)  # [B, H, 2, T/2]
    full = np.empty((B, H, T), np.float16)
    full[:, :, 0::2] = po[:, :, 0, :]
    full[:, :, 1::2] = po[:, :, 1, :]
    return np.ascontiguousarray(full.transpose(0, 2, 1)).astype(np.float32)
